# revision 18
# baseline (speedup 1.0000x reference)
"""Trainium2 Bass kernel for nn_DifferentiableLattice (gnn_message_passing).

Reference computation (per step, 9 steps):
    m = max(state)                         # global over (B, N)
    state = state @ P.T
    state = state * angle_factor * decay
    state = sigmoid(2*state - 1) * max(m, 0.1)
then out = sum_t softmax(step_weights)[t] * state_t   (incl. state_0 = x)

Kernel strategy (8 NeuronCores, data-parallel over batch):
  * Host precomputes W2 = 2*decay*diag(angle_factor) @ P  (512x512) and the
    softmax weights w[t]; shards x row-wise into 8 x [2048, 512].
  * On-chip state is the *unscaled* sigmoid output s~_t in float32r (TF32-like
    PE dtype: full matmul rate at N>=256, ~5e-5 matmul rel err vs bf16's
    ~8e-4), kept transposed [cells(part), batch(free)] so each step's matmul
    output layout feeds the next step's matmul directly:
        raw_t   = W2 @ s~_{t-1}                  (TensorE f32r, fp32 psum)
        s~_t    = sigmoid(c_{t-2} * raw_t - 1)   (ScalarE; runtime AP scale,
                                                  writes f32 scratch)
        acc    += (w_t * c_{t-1}) * s~_t         (VectorE scalar_tensor_tensor,
                                                  f32 source for accuracy)
        st_t    = round_f32r(s~_t) + pmax        (VectorE tensor_scalar with
                                                  accum_out=max)
    c_t = max(c_{t-1} * gmax(s~_t), 0.1); gmax is the global max across all
    8 shards: gpsimd partition_all_reduce + one tiny AllReduce(max) collective
    per step, overlapped with the following step's matmuls (the c consumer
    is two steps downstream).
  * x -> x^T and acc -> out transposes use PE identity-matmul transposes,
    pipelined through [128,1024] PSUM tiles (4-buffer rotation).
"""

import os
import sys

import numpy as np

sys.path.insert(0, "/opt/trn_rl_repo")

from contextlib import ExitStack

import concourse.bacc as bacc
import concourse.bass as bass
import concourse.bass_isa as bass_isa
import concourse.mybir as mybir
import concourse.tile as tile
from concourse.bass_utils import run_bass_kernel_spmd

F32 = mybir.dt.float32
BF16 = mybir.dt.bfloat16
F32R = mybir.dt.float32r
ST_DT = F32R
ALU = mybir.AluOpType
AX = mybir.AxisListType
ACTF = mybir.ActivationFunctionType

N_CELLS = 512
BATCH = 16384
N_CORES = 8
BSH = BATCH // N_CORES          # 2048 batch rows per core
KT = N_CELLS // 128             # 4 cell partition-tiles
NB = BSH // 512                 # 4 batch chunks of 512 (psum bank width)
NBT = BSH // 128                # 16 batch partition-tiles

LAST_RESULTS = None             # test harness peeks at this for profiling


def _host_prep(adjacency, std_devs, split_probs, join_probs, bounce_angles,
               step_weights, decay_rate, n_steps):
    """Replicate the reference's parameter preprocessing in float64."""
    adjacency = np.asarray(adjacency, np.float64)
    std_devs = np.asarray(std_devs, np.float64)
    split_probs = np.asarray(split_probs, np.float64)
    join_probs = np.asarray(join_probs, np.float64)
    bounce_angles = np.asarray(bounce_angles, np.float64)
    step_weights = np.asarray(step_weights, np.float64)
    decay_rate = np.asarray(decay_rate, np.float64)

    max_steps = step_weights.shape[0]
    actual_steps = min(int(n_steps), max_steps)
    # torch.clamp(x, min=2.0, max=0.99) saturates at 0.99
    decay = float(np.minimum(np.maximum(decay_rate, 2.0), 0.99)[0])

    from scipy.special import erf
    threshold = 0.5
    s = np.maximum(np.abs(std_devs), 2.0)
    straight = erf(threshold / (s * np.sqrt(2.0)))
    sp = np.clip(split_probs, 0.0, 1.0)
    jp = np.clip(join_probs, 0.0, 1.0)
    self_retention = straight * 0.3 * (1.0 - sp * 0.5)
    spread_factor = (1.0 - straight + sp * 0.3)[:, None]
    join_boost = (1.0 + jp * 0.5)[None, :]
    neighbor_spread = adjacency * spread_factor * join_boost
    prop = np.diag(self_retention) + neighbor_spread * 0.7
    prop = prop / np.clip(prop.sum(axis=1, keepdims=True), 1e-6, None)

    ang = np.clip(bounce_angles, 0.0, 2.0)
    angle_factor = 0.5 + 0.5 * np.cos(ang.mean(axis=1))

    W2 = (2.0 * decay) * (angle_factor[:, None] * prop)     # (N, N) rows j
    sw = step_weights[: actual_steps + 1]
    sw = sw - sw.max()
    e = np.exp(sw)
    w = e / e.sum()                                          # softmax weights

    return actual_steps, np.ascontiguousarray(W2.T), w.astype(np.float64)


def _build_program(steps, w):
    """Emit the SPMD Tile program for `steps` propagation steps.

    w: numpy float array of length steps+1 (softmax history weights).
    """
    nc = bacc.Bacc("TRN2", target_bir_lowering=False, debug=False,
                   num_devices=N_CORES)

    xhi_d = nc.dram_tensor("xhi", [BSH, N_CELLS], BF16, kind="ExternalInput")
    xlo_d = nc.dram_tensor("xlo", [BSH, N_CELLS], BF16, kind="ExternalInput")
    w2t_d = nc.dram_tensor("w2t", [N_CELLS, N_CELLS], F32, kind="ExternalInput")
    id_d = nc.dram_tensor("ident", [128, 128], F32, kind="ExternalInput")
    out_d = nc.dram_tensor("out", [BSH, N_CELLS], F32, kind="ExternalOutput")

    groups = [list(range(N_CORES))]

    with tile.TileContext(nc) as tc, ExitStack() as ctx:
        const = ctx.enter_context(tc.tile_pool(name="const", bufs=1))
        ldp = ctx.enter_context(tc.tile_pool(name="ldp", bufs=2))
        stg = ctx.enter_context(tc.tile_pool(name="stg", bufs=2))
        outp = ctx.enter_context(tc.tile_pool(name="outp", bufs=4))
        small = ctx.enter_context(tc.tile_pool(name="small", bufs=3))
        psp = ctx.enter_context(tc.tile_pool(name="psp", bufs=4, space="PSUM"))
        ccd = ctx.enter_context(tc.tile_pool(name="ccd", bufs=3, space="DRAM"))

        ident = const.tile([128, 128], F32, tag="ident", name="ident")
        nc.sync.dma_start(ident[:], id_d[:])

        neg1 = const.tile([128, 1], F32, tag="neg1", name="neg1")
        nc.vector.memset(neg1[:], -1.0)

        w2t = [const.tile([128, N_CELLS], ST_DT, tag=f"w2t{k}", name=f"w2t{k}")
               for k in range(KT)]
        for k in range(KT):
            wstg = ldp.tile([128, N_CELLS], F32, tag="wstg", name="wstg")
            nc.sync.dma_start(wstg[:], w2t_d[k * 128:(k + 1) * 128, :])
            nc.vector.tensor_copy(w2t[k][:], wstg[:])

        # double-buffered transposed state s~ [cell(part), batch(free)], bf16
        st = [[const.tile([128, BSH], ST_DT, tag=f"st{p}{k}", name=f"st{p}{k}")
               for k in range(KT)] for p in range(2)]
        acc = [const.tile([128, BSH], F32, tag=f"acc{j}", name=f"acc{j}")
               for j in range(KT)]
        # sized so a full step's f32 sigmoid tiles can wait for the collective
        # that gates their accumulate, without stalling the next step's ACTs
        sf32p = ctx.enter_context(tc.tile_pool(name="sf32p", bufs=7))

        # ---------------- prologue: DMA-xbar transpose of bf16 hi/lo halves
        # of x, recombined exactly on VectorE into f32r state tiles.
        for k in range(KT):
            thi = stg.tile([128, BSH], BF16, tag="thi", name="thi")
            tlo = stg.tile([128, BSH], BF16, tag="tlo", name="tlo")
            for r in range(NB):
                nc.sync.dma_start_transpose(
                    thi[:, r * 512:(r + 1) * 512],
                    xhi_d[r * 512:(r + 1) * 512, k * 128:(k + 1) * 128],
                )
                nc.sync.dma_start_transpose(
                    tlo[:, r * 512:(r + 1) * 512],
                    xlo_d[r * 512:(r + 1) * 512, k * 128:(k + 1) * 128],
                )
            nc.vector.tensor_add(st[0][k][:], thi[:], tlo[:])

        # acc init: acc_j = w0 * x^T_j ; also local max of state_0 = x
        pmt = small.tile([128, KT], F32, tag="pmt", name="pmt")
        for j in range(KT):
            nc.scalar.mul(acc[j][:], st[0][j][:], float(w[0]))
            nc.vector.reduce_max(pmt[:, j:j + 1], st[0][j][:], axis=AX.X)

        def launch_allreduce(pmt_tile):
            pm = small.tile([128, 1], F32, tag="pm", name="pm")
            nc.vector.reduce_max(pm[:], pmt_tile[:], axis=AX.X)
            pmr = small.tile([128, 1], F32, tag="pmr", name="pmr")
            nc.gpsimd.partition_all_reduce(pmr[:], pm[:], channels=128,
                                           reduce_op=bass_isa.ReduceOp.max)
            cin = small.tile([1, 8], F32, tag="cin", name="cin")
            nc.vector.memset(cin[:], 0.0)
            nc.vector.tensor_copy(cin[0:1, 0:1], pmr[0:1, 0:1])
            cc_in = ccd.tile([1, 8], F32, tag="ccin", name="ccin")
            cc_out = ccd.tile([1, 8], F32, tag="ccout", name="ccout")
            nc.gpsimd.dma_start(cc_in[:], cin[:])
            nc.gpsimd.collective_compute(
                "AllReduce", ALU.max, replica_groups=groups,
                ins=[cc_in.opt()], outs=[cc_out.opt()],
            )
            gm = small.tile([1, 8], F32, tag="gm", name="gm")
            nc.gpsimd.dma_start(gm[:], cc_out[:])
            return gm

        gm_prev = launch_allreduce(pmt)         # global max of state_0
        cvec_prev = None                        # c_{t-2} replicated [128,1]

        # ---------------- main steps
        for t in range(1, steps + 1):
            ph, prev = t % 2, (t - 1) % 2

            act_scale = cvec_prev               # c_{t-2}; None for t=1

            # consume gm_{t-1}: c_{t-1} = max(c_{t-2}*gmax, 0.1); coef_t = w_t*c_{t-1}
            gmb = small.tile([128, 1], F32, tag="gmb", name="gmb")
            nc.gpsimd.partition_broadcast(gmb[:], gm_prev[0:1, 0:1], channels=128)
            cvec = small.tile([128, 1], F32, tag="cvec", name="cvec", bufs=4)
            if cvec_prev is None:
                nc.vector.tensor_scalar(cvec[:], gmb[:], 0.1, None, op0=ALU.max)
            else:
                nc.vector.tensor_scalar(cvec[:], gmb[:], cvec_prev[:, 0:1], 0.1,
                                        op0=ALU.mult, op1=ALU.max)
            coef = small.tile([128, 1], F32, tag="coef", name="coef")
            nc.vector.tensor_scalar(coef[:], cvec[:], float(w[t]), None,
                                    op0=ALU.mult)

            pmt = (small.tile([128, KT], F32, tag="pmt", name="pmt")
                   if t < steps else None)
            sfs = []
            for j in range(KT):
                sf = sf32p.tile([128, BSH], F32, tag="sf", name="sf")
                for h in range(2):
                    ps = psp.tile([128, 1024], F32, tag="ps", name="ps")
                    for b2 in range(2):
                        b = h * 2 + b2
                        for k in range(KT):
                            nc.tensor.matmul(
                                ps[:, b2 * 512:(b2 + 1) * 512],
                                w2t[k][:, j * 128:(j + 1) * 128],
                                st[prev][k][:, b * 512:(b + 1) * 512],
                                start=(k == 0), stop=(k == KT - 1),
                            )
                    # f32 sigmoid output (feeds the accumulate exactly)
                    nc.scalar.activation(
                        sf[:, h * 1024:(h + 1) * 1024], ps[:], ACTF.Sigmoid,
                        bias=neg1[:, 0:1],
                        scale=(act_scale[:, 0:1] if act_scale is not None else 1.0),
                    )
                sfs.append(sf)
                # bf16 cast for the next matmul; rides the per-partition max.
                # Last step needs neither (no further matmul, no further max).
                if pmt is not None:
                    nc.vector.tensor_scalar(
                        st[ph][j][:], sf[:], 1.0, None,
                        op0=ALU.mult, op1=ALU.max,
                        accum_out=pmt[:, j:j + 1],
                    )

            gm_next = launch_allreduce(pmt) if pmt is not None else None

            # acc_j += coef_t * s~_t (fused multiply-add on VectorE, f32 source)
            for j in range(KT):
                nc.vector.scalar_tensor_tensor(
                    acc[j][:], sfs[j][:], coef[:, 0:1], acc[j][:],
                    op0=ALU.mult, op1=ALU.add,
                )

            gm_prev = gm_next
            cvec_prev = cvec

        # ---------------- epilogue: transpose acc -> out rows, store
        for i0 in range(0, NBT, 4):
            for dh in range(2):
                ps = psp.tile([128, 1024], F32, tag="ps", name="ps")
                for d2 in range(2):
                    di = dh * 2 + d2
                    for j in range(KT):
                        nc.tensor.transpose(
                            ps[:, d2 * 512 + j * 128: d2 * 512 + (j + 1) * 128],
                            acc[j][:, (i0 + di) * 128:(i0 + di + 1) * 128],
                            ident[:],
                        )
                for d2 in range(2):
                    di = dh * 2 + d2
                    ot = outp.tile([128, N_CELLS], F32, tag="ot", name="ot")
                    nc.scalar.copy(ot[:], ps[:, d2 * 512:(d2 + 1) * 512])
                    nc.sync.dma_start(out_d[(i0 + di) * 128:(i0 + di + 1) * 128, :],
                                      ot[:])

    nc.compile()
    return nc


def kernel(initial_activations, adjacency, std_devs, split_probs, join_probs,
           bounce_angles, step_weights, decay_rate, n_steps):
    global LAST_RESULTS
    x = np.ascontiguousarray(np.asarray(initial_activations, np.float32))
    steps, w2t_np, w = _host_prep(adjacency, std_devs, split_probs, join_probs,
                                  bounce_angles, step_weights, decay_rate,
                                  n_steps)
    if steps == 0:
        return (x * np.float32(1.0)).astype(np.float32)

    nc = _build_program(steps, w)

    import ml_dtypes
    w2tf = w2t_np.astype(np.float32)
    ident = np.eye(128, dtype=np.float32)
    xhi = x.astype(ml_dtypes.bfloat16)
    xlo = (x - xhi.astype(np.float32)).astype(ml_dtypes.bfloat16)
    in_maps = [
        {"xhi": xhi[c * BSH:(c + 1) * BSH], "xlo": xlo[c * BSH:(c + 1) * BSH],
         "w2t": w2tf, "ident": ident}
        for c in range(N_CORES)
    ]
    res = run_bass_kernel_spmd(
        nc, in_maps, core_ids=list(range(N_CORES)),
        trace=bool(os.environ.get("BASS_TRACE")),
    )
    LAST_RESULTS = res
    out = np.concatenate([res.results[c]["out"] for c in range(N_CORES)], axis=0)
    return np.ascontiguousarray(out.astype(np.float32))


if __name__ == "__main__":
    rng = np.random.default_rng(0)
    ins = {
        "initial_activations": rng.random((BATCH, N_CELLS), np.float32),
        "adjacency": (rng.random((N_CELLS, N_CELLS)) < 6.0 / 512).astype(np.float32),
        "std_devs": rng.standard_normal(N_CELLS).astype(np.float32),
        "split_probs": rng.random(N_CELLS).astype(np.float32),
        "join_probs": rng.random(N_CELLS).astype(np.float32),
        "bounce_angles": (rng.random((N_CELLS, 6)) * 2).astype(np.float32),
        "step_weights": rng.standard_normal(10).astype(np.float32),
        "decay_rate": np.ones(1, np.float32),
        "n_steps": 9,
    }
    o = kernel(**ins)
    print("out", o.shape, o.dtype, float(o.mean()))


# revision 19
# speedup vs baseline: 1.0746x; 1.0746x over previous
"""Trainium2 Bass kernel for nn_DifferentiableLattice (gnn_message_passing).

Reference computation (per step, 9 steps):
    m = max(state)                         # global over (B, N)
    state = state @ P.T
    state = state * angle_factor * decay
    state = sigmoid(2*state - 1) * max(m, 0.1)
then out = sum_t softmax(step_weights)[t] * state_t   (incl. state_0 = x)

Kernel strategy (8 NeuronCores, data-parallel over batch):
  * Host precomputes W2 = 2*decay*diag(angle_factor) @ P  (512x512) and the
    softmax weights w[t]; shards x row-wise into 8 x [2048, 512].
  * On-chip state is the *unscaled* sigmoid output s~_t in float32r (TF32-like
    PE dtype: full matmul rate at N>=256, ~5e-5 matmul rel err vs bf16's
    ~8e-4), kept transposed [cells(part), batch(free)] so each step's matmul
    output layout feeds the next step's matmul directly:
        raw_t   = W2 @ s~_{t-1}                  (TensorE f32r, fp32 psum)
        s~_t    = sigmoid(c_{t-2} * raw_t - 1)   (ScalarE; runtime AP scale,
                                                  writes f32 scratch)
        acc    += (w_t * c_{t-1}) * s~_t         (VectorE scalar_tensor_tensor,
                                                  f32 source for accuracy)
        st_t    = round_f32r(s~_t) + pmax        (VectorE tensor_scalar with
                                                  accum_out=max)
    c_t = max(c_{t-1} * gmax(s~_t), 0.1); gmax is the global max across all
    8 shards: gpsimd partition_all_reduce + one tiny AllReduce(max) collective
    per step, overlapped with the following step's matmuls (the c consumer
    is two steps downstream).
  * x -> x^T and acc -> out transposes use PE identity-matmul transposes,
    pipelined through [128,1024] PSUM tiles (4-buffer rotation).
"""

import os
import sys

import numpy as np

sys.path.insert(0, "/opt/trn_rl_repo")

from contextlib import ExitStack

import concourse.bacc as bacc
import concourse.bass as bass
import concourse.bass_isa as bass_isa
import concourse.mybir as mybir
import concourse.tile as tile
from concourse.bass_utils import run_bass_kernel_spmd

F32 = mybir.dt.float32
BF16 = mybir.dt.bfloat16
F32R = mybir.dt.float32r
ST_DT = F32R
ALU = mybir.AluOpType
AX = mybir.AxisListType
ACTF = mybir.ActivationFunctionType

N_CELLS = 512
BATCH = 16384
N_CORES = 8
BSH = BATCH // N_CORES          # 2048 batch rows per core
KT = N_CELLS // 128             # 4 cell partition-tiles
NB = BSH // 512                 # 4 batch chunks of 512 (psum bank width)
NBT = BSH // 128                # 16 batch partition-tiles

LAST_RESULTS = None             # test harness peeks at this for profiling


def _host_prep(adjacency, std_devs, split_probs, join_probs, bounce_angles,
               step_weights, decay_rate, n_steps):
    """Replicate the reference's parameter preprocessing in float64."""
    adjacency = np.asarray(adjacency, np.float64)
    std_devs = np.asarray(std_devs, np.float64)
    split_probs = np.asarray(split_probs, np.float64)
    join_probs = np.asarray(join_probs, np.float64)
    bounce_angles = np.asarray(bounce_angles, np.float64)
    step_weights = np.asarray(step_weights, np.float64)
    decay_rate = np.asarray(decay_rate, np.float64)

    max_steps = step_weights.shape[0]
    actual_steps = min(int(n_steps), max_steps)
    # torch.clamp(x, min=2.0, max=0.99) saturates at 0.99
    decay = float(np.minimum(np.maximum(decay_rate, 2.0), 0.99)[0])

    from scipy.special import erf
    threshold = 0.5
    s = np.maximum(np.abs(std_devs), 2.0)
    straight = erf(threshold / (s * np.sqrt(2.0)))
    sp = np.clip(split_probs, 0.0, 1.0)
    jp = np.clip(join_probs, 0.0, 1.0)
    self_retention = straight * 0.3 * (1.0 - sp * 0.5)
    spread_factor = (1.0 - straight + sp * 0.3)[:, None]
    join_boost = (1.0 + jp * 0.5)[None, :]
    neighbor_spread = adjacency * spread_factor * join_boost
    prop = np.diag(self_retention) + neighbor_spread * 0.7
    prop = prop / np.clip(prop.sum(axis=1, keepdims=True), 1e-6, None)

    ang = np.clip(bounce_angles, 0.0, 2.0)
    angle_factor = 0.5 + 0.5 * np.cos(ang.mean(axis=1))

    W2 = (2.0 * decay) * (angle_factor[:, None] * prop)     # (N, N) rows j
    sw = step_weights[: actual_steps + 1]
    sw = sw - sw.max()
    e = np.exp(sw)
    w = e / e.sum()                                          # softmax weights

    return actual_steps, np.ascontiguousarray(W2.T), w.astype(np.float64)


def _build_program(steps, w):
    """Emit the SPMD Tile program for `steps` propagation steps.

    w: numpy float array of length steps+1 (softmax history weights).
    """
    nc = bacc.Bacc("TRN2", target_bir_lowering=False, debug=False,
                   num_devices=N_CORES)

    x_d = nc.dram_tensor("x", [BSH, N_CELLS], F32, kind="ExternalInput")
    w2t_d = nc.dram_tensor("w2t", [N_CELLS, N_CELLS], F32, kind="ExternalInput")
    id_d = nc.dram_tensor("ident", [128, 128], F32, kind="ExternalInput")
    out_d = nc.dram_tensor("out", [BSH, N_CELLS], F32, kind="ExternalOutput")

    groups = [list(range(N_CORES))]

    with tile.TileContext(nc) as tc, ExitStack() as ctx:
        const = ctx.enter_context(tc.tile_pool(name="const", bufs=1))
        ldp = ctx.enter_context(tc.tile_pool(name="ldp", bufs=8))
        outp = ctx.enter_context(tc.tile_pool(name="outp", bufs=4))
        small = ctx.enter_context(tc.tile_pool(name="small", bufs=3))
        psp = ctx.enter_context(tc.tile_pool(name="psp", bufs=4, space="PSUM"))
        ccd = ctx.enter_context(tc.tile_pool(name="ccd", bufs=3, space="DRAM"))

        ident = const.tile([128, 128], F32, tag="ident", name="ident")
        nc.sync.dma_start(ident[:], id_d[:])

        neg1 = const.tile([128, 1], F32, tag="neg1", name="neg1")
        nc.vector.memset(neg1[:], -1.0)

        w2t = [const.tile([128, N_CELLS], ST_DT, tag=f"w2t{k}", name=f"w2t{k}")
               for k in range(KT)]
        for k in range(KT):
            wstg = ldp.tile([128, N_CELLS], F32, tag="wstg", name="wstg")
            nc.sync.dma_start(wstg[:], w2t_d[k * 128:(k + 1) * 128, :])
            nc.vector.tensor_copy(w2t[k][:], wstg[:])

        # double-buffered transposed state s~ [cell(part), batch(free)], bf16
        st = [[const.tile([128, BSH], ST_DT, tag=f"st{p}{k}", name=f"st{p}{k}")
               for k in range(KT)] for p in range(2)]
        acc = [const.tile([128, BSH], F32, tag=f"acc{j}", name=f"acc{j}")
               for j in range(KT)]
        # sized so a full step's f32 sigmoid tiles can wait for the collective
        # that gates their accumulate, without stalling the next step's ACTs
        sf32p = ctx.enter_context(tc.tile_pool(name="sf32p", bufs=6))

        # ---------------- prologue: load x, PE-transpose into st[0] (f32->f32r)
        for i0 in range(0, NBT, 4):
            xt = []
            for di in range(4):
                t = ldp.tile([128, N_CELLS], F32, tag="xld", name="xld")
                nc.sync.dma_start(t[:], x_d[(i0 + di) * 128:(i0 + di + 1) * 128, :])
                xt.append(t)
            for kh in range(2):
                ps = psp.tile([128, 1024], F32, tag="ps", name="ps")
                for k2 in range(2):
                    k = kh * 2 + k2
                    for di in range(4):
                        nc.tensor.transpose(
                            ps[:, k2 * 512 + di * 128: k2 * 512 + (di + 1) * 128],
                            xt[di][:, k * 128:(k + 1) * 128],
                            ident[:],
                        )
                for k2 in range(2):
                    k = kh * 2 + k2
                    nc.scalar.copy(st[0][k][:, i0 * 128: i0 * 128 + 512],
                                   ps[:, k2 * 512:(k2 + 1) * 512])

        # acc init: acc_j = w0 * x^T_j ; also local max of state_0 = x
        pmt = small.tile([128, KT], F32, tag="pmt", name="pmt")
        for j in range(KT):
            nc.scalar.mul(acc[j][:], st[0][j][:], float(w[0]))
            nc.vector.reduce_max(pmt[:, j:j + 1], st[0][j][:], axis=AX.X)

        def launch_allreduce(pmt_tile):
            pm = small.tile([128, 1], F32, tag="pm", name="pm")
            nc.vector.reduce_max(pm[:], pmt_tile[:], axis=AX.X)
            pmr = small.tile([128, 1], F32, tag="pmr", name="pmr")
            nc.gpsimd.partition_all_reduce(pmr[:], pm[:], channels=128,
                                           reduce_op=bass_isa.ReduceOp.max)
            cin = small.tile([1, 8], F32, tag="cin", name="cin")
            nc.vector.memset(cin[:], 0.0)
            nc.vector.tensor_copy(cin[0:1, 0:1], pmr[0:1, 0:1])
            cc_in = ccd.tile([1, 8], F32, tag="ccin", name="ccin")
            cc_out = ccd.tile([1, 8], F32, tag="ccout", name="ccout")
            nc.gpsimd.dma_start(cc_in[:], cin[:])
            nc.gpsimd.collective_compute(
                "AllReduce", ALU.max, replica_groups=groups,
                ins=[cc_in.opt()], outs=[cc_out.opt()],
            )
            gm = small.tile([1, 8], F32, tag="gm", name="gm")
            nc.gpsimd.dma_start(gm[:], cc_out[:])
            return gm

        gm_prev = launch_allreduce(pmt)         # global max of state_0
        cvec_prev = None                        # c_{t-2} replicated [128,1]

        # ---------------- main steps
        for t in range(1, steps + 1):
            ph, prev = t % 2, (t - 1) % 2

            act_scale = cvec_prev               # c_{t-2}; None for t=1

            # consume gm_{t-1}: c_{t-1} = max(c_{t-2}*gmax, 0.1); coef_t = w_t*c_{t-1}
            gmb = small.tile([128, 1], F32, tag="gmb", name="gmb")
            nc.gpsimd.partition_broadcast(gmb[:], gm_prev[0:1, 0:1], channels=128)
            cvec = small.tile([128, 1], F32, tag="cvec", name="cvec", bufs=4)
            if cvec_prev is None:
                nc.vector.tensor_scalar(cvec[:], gmb[:], 0.1, None, op0=ALU.max)
            else:
                nc.vector.tensor_scalar(cvec[:], gmb[:], cvec_prev[:, 0:1], 0.1,
                                        op0=ALU.mult, op1=ALU.max)
            coef = small.tile([128, 1], F32, tag="coef", name="coef")
            nc.vector.tensor_scalar(coef[:], cvec[:], float(w[t]), None,
                                    op0=ALU.mult)

            pmt = (small.tile([128, KT], F32, tag="pmt", name="pmt")
                   if t < steps else None)
            sfs = []
            for j in range(KT):
                sf = sf32p.tile([128, BSH], F32, tag="sf", name="sf")
                for h in range(2):
                    ps = psp.tile([128, 1024], F32, tag="ps", name="ps")
                    for b2 in range(2):
                        b = h * 2 + b2
                        for k in range(KT):
                            nc.tensor.matmul(
                                ps[:, b2 * 512:(b2 + 1) * 512],
                                w2t[k][:, j * 128:(j + 1) * 128],
                                st[prev][k][:, b * 512:(b + 1) * 512],
                                start=(k == 0), stop=(k == KT - 1),
                            )
                    # f32 sigmoid output (feeds the accumulate exactly)
                    nc.scalar.activation(
                        sf[:, h * 1024:(h + 1) * 1024], ps[:], ACTF.Sigmoid,
                        bias=neg1[:, 0:1],
                        scale=(act_scale[:, 0:1] if act_scale is not None else 1.0),
                    )
                sfs.append(sf)
                # bf16 cast for the next matmul; rides the per-partition max.
                # Last step needs neither (no further matmul, no further max).
                if pmt is not None:
                    nc.vector.tensor_scalar(
                        st[ph][j][:], sf[:], 1.0, None,
                        op0=ALU.mult, op1=ALU.max,
                        accum_out=pmt[:, j:j + 1],
                    )

            gm_next = launch_allreduce(pmt) if pmt is not None else None

            # acc_j += coef_t * s~_t (fused multiply-add on VectorE, f32 source)
            for j in range(KT):
                nc.vector.scalar_tensor_tensor(
                    acc[j][:], sfs[j][:], coef[:, 0:1], acc[j][:],
                    op0=ALU.mult, op1=ALU.add,
                )

            gm_prev = gm_next
            cvec_prev = cvec

        # ---------------- epilogue: transpose acc -> out rows, store
        for i0 in range(0, NBT, 4):
            for dh in range(2):
                ps = psp.tile([128, 1024], F32, tag="ps", name="ps")
                for d2 in range(2):
                    di = dh * 2 + d2
                    for j in range(KT):
                        nc.tensor.transpose(
                            ps[:, d2 * 512 + j * 128: d2 * 512 + (j + 1) * 128],
                            acc[j][:, (i0 + di) * 128:(i0 + di + 1) * 128],
                            ident[:],
                        )
                for d2 in range(2):
                    di = dh * 2 + d2
                    ot = outp.tile([128, N_CELLS], F32, tag="ot", name="ot")
                    nc.scalar.copy(ot[:], ps[:, d2 * 512:(d2 + 1) * 512])
                    nc.sync.dma_start(out_d[(i0 + di) * 128:(i0 + di + 1) * 128, :],
                                      ot[:])

    nc.compile()
    return nc


def kernel(initial_activations, adjacency, std_devs, split_probs, join_probs,
           bounce_angles, step_weights, decay_rate, n_steps):
    global LAST_RESULTS
    x = np.ascontiguousarray(np.asarray(initial_activations, np.float32))
    steps, w2t_np, w = _host_prep(adjacency, std_devs, split_probs, join_probs,
                                  bounce_angles, step_weights, decay_rate,
                                  n_steps)
    if steps == 0:
        return (x * np.float32(1.0)).astype(np.float32)

    nc = _build_program(steps, w)

    w2tf = w2t_np.astype(np.float32)
    ident = np.eye(128, dtype=np.float32)
    in_maps = [
        {"x": x[c * BSH:(c + 1) * BSH], "w2t": w2tf, "ident": ident}
        for c in range(N_CORES)
    ]
    res = run_bass_kernel_spmd(
        nc, in_maps, core_ids=list(range(N_CORES)),
        trace=bool(os.environ.get("BASS_TRACE")),
    )
    LAST_RESULTS = res
    out = np.concatenate([res.results[c]["out"] for c in range(N_CORES)], axis=0)
    return np.ascontiguousarray(out.astype(np.float32))


if __name__ == "__main__":
    rng = np.random.default_rng(0)
    ins = {
        "initial_activations": rng.random((BATCH, N_CELLS), np.float32),
        "adjacency": (rng.random((N_CELLS, N_CELLS)) < 6.0 / 512).astype(np.float32),
        "std_devs": rng.standard_normal(N_CELLS).astype(np.float32),
        "split_probs": rng.random(N_CELLS).astype(np.float32),
        "join_probs": rng.random(N_CELLS).astype(np.float32),
        "bounce_angles": (rng.random((N_CELLS, 6)) * 2).astype(np.float32),
        "step_weights": rng.standard_normal(10).astype(np.float32),
        "decay_rate": np.ones(1, np.float32),
        "n_steps": 9,
    }
    o = kernel(**ins)
    print("out", o.shape, o.dtype, float(o.mean()))
